# revision 28
# baseline (speedup 1.0000x reference)
"""Cross-attention kernel for TRN2, SPMD over 8 NeuronCores.

Problem: B=8, SQ=4096, SKV=77, D_EMBED=1024, D_CROSS=768, H=16, DH=64.
  q = x @ wq + bq ; k = y @ wk + bk ; v = y @ wv + bv
  out = softmax(q k^T / 8) v @ wo + bo

Sharding: pure data-parallel over batch (1 batch element per core, no
collectives). Host pre-transposes x and y per core so the device kernel
keeps every tensor feature-major (contraction dim on partitions) until the
O-projection, which uses attnout^T as the stationary operand to emit the
output in natural row-major layout.

Perf structure: the PE instruction stream is kept dense so the HAM
clock-gate stays at 8/8 (2.4 GHz) instead of oscillating:
  - Solid Q-proj blocks (64 back-to-back matmuls) anchor each iteration;
    the ACT-bound attention phase of chunk c is back-filled with O-proj
    tiles of chunk c-1 (and with Q-proj(1) tiles for chunk 0, which has no
    prior O-proj), a few of them right after the softmax reciprocal to
    bridge its DVE latency.
  - Bias adds burn no PE matmuls: q-bias rides the scalar-engine
    PSUM->SBUF copy as a per-partition bias, o-bias is a DVE tensor_add
    against a host-pre-broadcast [128, D] bias tile.
  - The 1/sum row-broadcast is one [16,128]-selector matmul per head pair.
  - Startup DMAs are split across the two HWDGE rings (sync + scalar) so
    K-proj / Q-proj(0) / V-proj inputs land in parallel.

Compute dtype: bf16 operands (host-cast), fp32 PSUM accumulation, fp32 out.

Softmax is computed without max-subtraction (scores are O(5) for this
problem class; exp stays comfortably inside fp32/bf16 range):
  scoresT[s,q] = k'_h @ q_h^T with k' = (k + bk)/8 folded at k-projection
  e = exp(scoresT)  (bf16)
  r = 1 / (sel16^T @ e)         per-head [16, SQ] via PE column-sum
  aoT[d,q] = (v_h^T @ e) * rb   with rb = selb^T @ r (PE broadcast)
  out[q,:] = aoT^T @ wo + bo    (aoT tiles as stationary operand)
"""

import numpy as np
import ml_dtypes

import concourse.bass as bass
import concourse.mybir as mybir
import concourse.tile as tile
from concourse import bacc
from concourse import bass_utils

F32 = mybir.dt.float32
BF16 = mybir.dt.bfloat16
AF = mybir.ActivationFunctionType

B = 8
SQ = 4096
SKV = 77
D = 1024
DC = 768
H = 16
DH = 64
KT = D // 128    # 8 embed k-tiles
KC = DC // 128   # 6 cross k-tiles
CT = D // 128    # 8 column tiles of the 1024-wide projections
CH = 512         # query chunk
NCH = SQ // CH   # 8 chunks
NQT = CH // 128  # 4 query 128-tiles per chunk

_CACHED = {}


def _build():
    nc = bacc.Bacc("TRN2", target_bir_lowering=False, debug=False, num_devices=B)

    # all host-pre-tiled to the device SBUF layout (contiguous DMAs)
    xt = nc.dram_tensor("xt", (128, KT * SQ), BF16, kind="ExternalInput")
    yt = nc.dram_tensor("yt", (128, KC * SKV), BF16, kind="ExternalInput")
    wq_d = nc.dram_tensor("wq", (128, KT * D), BF16, kind="ExternalInput")
    wk_d = nc.dram_tensor("wk", (128, KC * D), BF16, kind="ExternalInput")
    wv_d = nc.dram_tensor("wv", (128, KC * D), BF16, kind="ExternalInput")
    wo_d = nc.dram_tensor("wo", (128, KT * D), BF16, kind="ExternalInput")
    bqc_d = nc.dram_tensor("bqc", (128, CT), F32, kind="ExternalInput")
    bk8_d = nc.dram_tensor("bk8", (128, CT), F32, kind="ExternalInput")
    bv_d = nc.dram_tensor("bv", (1, D), BF16, kind="ExternalInput")
    bo128_d = nc.dram_tensor("bo128", (128, D), BF16, kind="ExternalInput")
    sel16_d = nc.dram_tensor("sel16", (SKV, H * 16), BF16, kind="ExternalInput")
    selb_d = nc.dram_tensor("selb", (16, D), BF16, kind="ExternalInput")
    out_d = nc.dram_tensor("out", (SQ, D), F32, kind="ExternalOutput")

    with tile.TileContext(nc) as tc:
        with (
            tc.tile_pool(name="consts", bufs=1) as consts,
            tc.tile_pool(name="wpool", bufs=1) as wpool,
            tc.tile_pool(name="xpool", bufs=2) as xpool,
            tc.tile_pool(name="qpool", bufs=2) as qpool,
            tc.tile_pool(name="epool", bufs=2) as epool,
            tc.tile_pool(name="rp", bufs=2) as rp,
            tc.tile_pool(name="rbpool", bufs=2) as rbpool,
            tc.tile_pool(name="aopool", bufs=2) as aopool,
            tc.tile_pool(name="opool", bufs=3) as opool,
            tc.tile_pool(name="pmm", bufs=2, space="PSUM") as pmm,
            tc.tile_pool(name="psc", bufs=2, space="PSUM") as psc,
            tc.tile_pool(name="ppv", bufs=2, space="PSUM") as ppv,
            tc.tile_pool(name="pnrm", bufs=2, space="PSUM") as pnrm,
        ):
            aoT_tiles = [None, None]
            xT_tiles = {}
            qT_tiles = {}

            def fetch_x(cc, engine=None):
                if cc >= NCH or cc in xT_tiles:
                    return
                t = xpool.tile([128, KT, CH], BF16, tag="xT", name="xT")
                (engine or nc.sync).dma_start(
                    t[:],
                    xt.ap().rearrange("p (kt q) -> p kt q", q=SQ)[
                        :, :, cc * CH:(cc + 1) * CH],
                )
                xT_tiles[cc] = t

            # ---- weights/constants on two parallel HWDGE rings ----
            # ring-sync: wk, xT0, wv, xT1, wo ; ring-act: yt/smalls, wq, rest
            wk_sb = wpool.tile([128, KC, D], BF16, tag="wk")
            nc.sync.dma_start(wk_sb[:], wk_d.ap())
            fetch_x(0)
            wv_sb = wpool.tile([128, KC, D], BF16, tag="wv")
            nc.sync.dma_start(wv_sb[:], wv_d.ap())
            fetch_x(1)
            wo_sb = wpool.tile([128, KT, D], BF16, tag="wo")
            nc.sync.dma_start(wo_sb[:], wo_d.ap())

            yt_sb = consts.tile([128, KC, SKV], BF16, tag="yt")
            nc.scalar.dma_start(yt_sb[:], yt.ap())
            bk8_sb = consts.tile([128, CT], F32, tag="bk8")
            nc.scalar.dma_start(bk8_sb[:], bk8_d.ap())
            wq_sb = wpool.tile([128, KT, D], BF16, tag="wq")
            nc.scalar.dma_start(wq_sb[:], wq_d.ap())
            bqc_sb = consts.tile([128, CT], F32, tag="bqc")
            nc.scalar.dma_start(bqc_sb[:], bqc_d.ap())
            bv_sb = consts.tile([1, D], BF16, tag="bv")
            nc.scalar.dma_start(bv_sb[:], bv_d.ap())
            sel16_sb = consts.tile([SKV, H * 16], BF16, tag="sel16")
            nc.scalar.dma_start(sel16_sb[:], sel16_d.ap())
            selb_sb = consts.tile([16, D], BF16, tag="selb")
            nc.scalar.dma_start(selb_sb[:], selb_d.ap())
            bo128 = consts.tile([128, D], BF16, tag="bo128")
            nc.scalar.dma_start(bo128[:], bo128_d.ap())

            ones77r = consts.tile([1, SKV], BF16, tag="ones77r")
            nc.vector.memset(ones77r[:], 1.0)

            # warm the PE HAM clock-gate while the weight DMAs are still in
            # flight: ~48 dummy matmuls on a memset tile (no DMA deps) so
            # K-proj / Q-proj(0) run at 2.4 GHz the moment their data lands.
            dwarm = consts.tile([128, CH], BF16, tag="dwarm")
            nc.vector.memset(dwarm[:], 0.0)
            pswarm = pmm.tile([128, CH], F32, tag="mm")
            for _ in range(60):
                nc.tensor.matmul(
                    pswarm[:], dwarm[:, 0:128], dwarm[:],
                    start=True, stop=True, skip_group_check=True,
                )

            kT_sb = consts.tile([128, CT, SKV], BF16, tag="kT")
            v_sb = consts.tile([SKV, H, DH], BF16, tag="v")

            def emit_qp_tile(cc, ct):
                """Q-proj column-tile ct of chunk cc: 8 matmuls + ACT bias-copy."""
                psq = pmm.tile([128, CH], F32, tag="mm")
                for kt in range(KT):
                    nc.tensor.matmul(
                        psq[:],
                        wq_sb[:, kt, ct * 128:(ct + 1) * 128],
                        xT_tiles[cc][:, kt, :],
                        start=(kt == 0),
                        stop=(kt == KT - 1),
                        skip_group_check=True,
                    )
                nc.scalar.activation(
                    qT_tiles[cc][:, ct, :],
                    psq[:],
                    AF.Identity,
                    bias=bqc_sb[:, ct:ct + 1],
                )
                if ct == CT - 1:
                    xT_tiles.pop(cc)

            def emit_qp_block(cc):
                qT_tiles[cc] = qpool.tile([128, CT, CH], BF16, tag="qT", name="qT")
                for ct in range(CT):
                    emit_qp_tile(cc, ct)

            def emit_op_tile(cc, t):
                """O-proj tile t=(qt*2+n) of chunk cc: 8 matmuls + DVE bias-add + DMA."""
                qt, n = t // 2, t % 2
                aoT_p = aoT_tiles[cc % 2]
                q0 = cc * CH
                pso = pmm.tile([128, CH], F32, tag="mm")
                for kt in range(KT):
                    nc.tensor.matmul(
                        pso[:],
                        aoT_p[:, kt, qt * 128:(qt + 1) * 128],
                        wo_sb[:, kt, n * 512:(n + 1) * 512],
                        start=(kt == 0),
                        stop=(kt == KT - 1),
                        skip_group_check=True,
                    )
                o_sb = opool.tile([128, CH], F32, tag="o")
                nc.vector.tensor_add(o_sb[:], pso[:], bo128[:, n * 512:(n + 1) * 512])
                nc.sync.dma_start(
                    out_d.ap()[q0 + qt * 128: q0 + (qt + 1) * 128,
                               n * 512:(n + 1) * 512],
                    o_sb[:],
                )

            # ---- k projection: kT[c, s] = sum_k wk[k, c] yT[k, s]; fold (.+bk)/8 ----
            for ct in range(CT):
                psk = pmm.tile([128, CH], F32, tag="mm")
                for kt in range(KC):
                    nc.tensor.matmul(
                        psk[:, 0:SKV],
                        wk_sb[:, kt, ct * 128:(ct + 1) * 128],
                        yt_sb[:, kt, :],
                        start=(kt == 0),
                        stop=(kt == KC - 1),
                    )
                nc.scalar.activation(
                    kT_sb[:, ct, :],
                    psk[:, 0:SKV],
                    AF.Identity,
                    scale=0.125,
                    bias=bk8_sb[:, ct:ct + 1],
                )

            # ---- v projection: v[s, c] = sum_k yT[k, s] wv[k, c] + bv[c] ----
            for n in range(2):
                psv = pmm.tile([128, CH], F32, tag="mm")
                for kt in range(KC):
                    nc.tensor.matmul(
                        psv[0:SKV, :],
                        yt_sb[:, kt, :],
                        wv_sb[:, kt, n * 512:(n + 1) * 512],
                        start=(kt == 0),
                        stop=False,
                    )
                nc.tensor.matmul(
                    psv[0:SKV, :],
                    ones77r[:],
                    bv_sb[0:1, n * 512:(n + 1) * 512],
                    start=False,
                    stop=True,
                )
                nc.vector.tensor_copy(
                    v_sb[:, n * 8:(n + 1) * 8, :], psv[0:SKV, :]
                )

            # ---- Q-proj(0) ----
            emit_qp_block(0)

            # ---- software-pipelined main loop ----
            # iter c: solid Q-proj(c+1) block (c>=1), then attention(c) with
            # fill tiles woven in: O-proj(c-1) tiles (or Q-proj(1) tiles for
            # c==0, which has no prior chunk).
            for c in range(NCH + 1):
                fills = []
                if c == 0:
                    # QP(1) tiles serve as the fills (no prior O-proj yet)
                    qT_tiles[1] = qpool.tile([128, CT, CH], BF16, tag="qT", name="qT")
                    fills = [("qp", 1, ct) for ct in range(CT)]
                elif c <= NCH - 1:
                    if c + 1 <= NCH - 1:
                        emit_qp_block(c + 1)
                    fills = [("op", c - 1, t) for t in range(8)]
                else:
                    for t in range(8):
                        emit_op_tile(c - 1, t)
                    break

                fill_i = [0]

                def drain(upto, _fills=fills, _i=fill_i):
                    while _i[0] < min(upto, len(_fills)):
                        kind, cc, idx = _fills[_i[0]]
                        if kind == "qp":
                            emit_qp_tile(cc, idx)
                        else:
                            emit_op_tile(cc, idx)
                        _i[0] += 1

                fetch_x(c + 2)

                # attention pass A: scores -> exp -> sum-collect [16, CH]
                e_ch = epool.tile([SKV, H, CH], BF16, tag="e")
                ps_sum = pnrm.tile([16, CH], F32, tag="nrm")
                for h in range(H):
                    pssc = psc.tile([SKV, CH], F32, tag="sc")
                    nc.tensor.matmul(
                        pssc[:],
                        kT_sb[(h % 2) * 64:(h % 2) * 64 + 64, h // 2, :],
                        qT_tiles[c][(h % 2) * 64:(h % 2) * 64 + 64, h // 2, :],
                        start=True, stop=True, skip_group_check=True,
                    )
                    nc.scalar.activation(e_ch[:, h, :], pssc[:], AF.Exp)
                    nc.tensor.matmul(
                        ps_sum[:], sel16_sb[:, h * 16:(h + 1) * 16], e_ch[:, h, :],
                        start=(h == 0), stop=(h == H - 1), skip_group_check=True,
                    )
                    if h in (3, 7, 11):
                        drain({3: 1, 7: 2, 11: 3}[h])

                qT_tiles.pop(c)
                r16f = rp.tile([16, CH], F32, tag="rf")
                nc.vector.reciprocal_approx_fast(r16f[:], ps_sum[:])
                r16 = rp.tile([16, CH], BF16, tag="r")
                with nc.allow_low_precision(reason="softmax recip in bf16"):
                    nc.vector.tensor_copy(r16[:], r16f[:])
                # fills here bridge the reciprocal's DVE latency so the PE
                # never idles long enough to trip the HAM re-throttle
                drain(6)

                # pass B: rb = selb^T @ r (broadcast 1/sum to 128 rows), PV,
                # normalize into aoT.
                aoT = aopool.tile([128, KT, CH], BF16, tag="aoT")
                aoT_tiles[c % 2] = aoT
                for hp in range(H // 2):
                    rb_ps = pnrm.tile([128, CH], F32, tag="nrm")
                    nc.tensor.matmul(
                        rb_ps[:],
                        selb_sb[:, hp * 128:(hp + 1) * 128],
                        r16[:],
                        start=True, stop=True, skip_group_check=True,
                    )
                    rb_sb = rbpool.tile([128, CH], F32, tag="rb")
                    nc.scalar.activation(rb_sb[:], rb_ps[:], AF.Identity)
                    pspv = ppv.tile([128, CH], F32, tag="pv")
                    for half in range(2):
                        h = 2 * hp + half
                        nc.tensor.matmul(
                            pspv[half * 64:(half + 1) * 64, :],
                            v_sb[:, h, :],
                            e_ch[:, h, :],
                            start=True, stop=True, skip_group_check=True,
                        )
                    nc.vector.tensor_mul(aoT[:, hp, :], pspv[:], rb_sb[:])
                    if hp in (2, 5):
                        drain({2: 7, 5: 8}[hp])

    nc.compile()
    return nc


def _get_nc():
    if "nc" not in _CACHED:
        _CACHED["nc"] = _build()
    return _CACHED["nc"]


def _prep_inmaps(x, y, wq, bq, wk, bk, wv, bv, wo, bo):
    x = np.asarray(x)
    y = np.asarray(y)
    bf = ml_dtypes.bfloat16

    def tile_w(w, kt):
        # [kt*128, n] -> [128, kt*n] in the device SBUF layout
        w = np.asarray(w)
        n = w.shape[1]
        return np.ascontiguousarray(
            w.reshape(kt, 128, n).transpose(1, 0, 2).reshape(128, kt * n)
        ).astype(bf)

    wq_b = tile_w(wq, KT)
    wk_b = tile_w(wk, KC)
    wv_b = tile_w(wv, KC)
    wo_b = tile_w(wo, KT)
    bv_b = np.asarray(bv).reshape(1, D).astype(bf)
    bo128 = np.ascontiguousarray(
        np.broadcast_to(np.asarray(bo).reshape(1, D), (128, D))).astype(bf)
    bqc = np.ascontiguousarray(
        np.asarray(bq).reshape(CT, 128).T).astype(np.float32)
    bk8 = np.ascontiguousarray(
        (np.asarray(bk).reshape(CT, 128) * 0.125).T).astype(np.float32)
    sel16 = np.zeros((SKV, H, 16), np.float32)
    sel16[:, np.arange(H), np.arange(16)] = 1.0
    sel16 = sel16.reshape(SKV, H * 16).astype(bf)
    selb = np.zeros((16, H // 2, 128), np.float32)
    for hp in range(H // 2):
        selb[2 * hp, hp, 0:64] = 1.0
        selb[2 * hp + 1, hp, 64:128] = 1.0
    selb = selb.reshape(16, D).astype(bf)

    in_maps = []
    for b in range(B):
        in_maps.append({
            "xt": tile_w(x[b].T, KT),
            "yt": tile_w(y[b].T, KC),
            "wq": wq_b, "wk": wk_b, "wv": wv_b, "wo": wo_b,
            "bqc": bqc, "bk8": bk8, "bv": bv_b, "bo128": bo128,
            "sel16": sel16, "selb": selb,
        })
    return in_maps


def kernel(x, y, wq, bq, wk, bk, wv, bv, wo, bo):
    in_maps = _prep_inmaps(x, y, wq, bq, wk, bk, wv, bv, wo, bo)
    nc = _get_nc()
    res = bass_utils.run_bass_kernel_spmd(nc, in_maps, core_ids=list(range(B)))
    out = np.stack([res.results[b]["out"] for b in range(B)], axis=0)
    return out.astype(np.float32)


# revision 29
# speedup vs baseline: 1.0301x; 1.0301x over previous
"""Cross-attention kernel for TRN2, SPMD over 8 NeuronCores.

Problem: B=8, SQ=4096, SKV=77, D_EMBED=1024, D_CROSS=768, H=16, DH=64.
  q = x @ wq + bq ; k = y @ wk + bk ; v = y @ wv + bv
  out = softmax(q k^T / 8) v @ wo + bo

Sharding: pure data-parallel over batch (1 batch element per core, no
collectives). Host pre-transposes x and y per core so the device kernel
keeps every tensor feature-major (contraction dim on partitions) until the
O-projection, which uses attnout^T as the stationary operand to emit the
output in natural row-major layout.

Perf structure: the PE instruction stream is kept dense so the HAM
clock-gate stays at 8/8 (2.4 GHz) instead of oscillating:
  - Solid Q-proj blocks (64 back-to-back matmuls) anchor each iteration;
    the ACT-bound attention phase of chunk c is back-filled with O-proj
    tiles of chunk c-1 (and with Q-proj(1) tiles for chunk 0, which has no
    prior O-proj), a few of them right after the softmax reciprocal to
    bridge its DVE latency.
  - Bias adds burn no PE matmuls: q-bias rides the scalar-engine
    PSUM->SBUF copy as a per-partition bias, o-bias is a DVE tensor_add
    against a host-pre-broadcast [128, D] bias tile.
  - The 1/sum row-broadcast is one [16,128]-selector matmul per head pair.
  - Startup DMAs are split across the two HWDGE rings (sync + scalar) so
    K-proj / Q-proj(0) / V-proj inputs land in parallel.

Compute dtype: bf16 operands (host-cast), fp32 PSUM accumulation, fp32 out.

Softmax is computed without max-subtraction (scores are O(5) for this
problem class; exp stays comfortably inside fp32/bf16 range):
  scoresT[s,q] = k'_h @ q_h^T with k' = (k + bk)/8 folded at k-projection
  e = exp(scoresT)  (bf16)
  r = 1 / (sel16^T @ e)         per-head [16, SQ] via PE column-sum
  aoT[d,q] = (v_h^T @ e) * rb   with rb = selb^T @ r (PE broadcast)
  out[q,:] = aoT^T @ wo + bo    (aoT tiles as stationary operand)
"""

import numpy as np
import ml_dtypes

import concourse.bass as bass
import concourse.mybir as mybir
import concourse.tile as tile
from concourse import bacc
from concourse import bass_utils

F32 = mybir.dt.float32
BF16 = mybir.dt.bfloat16
AF = mybir.ActivationFunctionType

B = 8
SQ = 4096
SKV = 77
D = 1024
DC = 768
H = 16
DH = 64
KT = D // 128    # 8 embed k-tiles
KC = DC // 128   # 6 cross k-tiles
CT = D // 128    # 8 column tiles of the 1024-wide projections
CH = 512         # query chunk
NCH = SQ // CH   # 8 chunks
NQT = CH // 128  # 4 query 128-tiles per chunk

_CACHED = {}


def _build():
    nc = bacc.Bacc("TRN2", target_bir_lowering=False, debug=False, num_devices=B)

    # all host-pre-tiled to the device SBUF layout (contiguous DMAs)
    xt = nc.dram_tensor("xt", (128, KT * SQ), BF16, kind="ExternalInput")
    yt = nc.dram_tensor("yt", (128, KC * SKV), BF16, kind="ExternalInput")
    wq_d = nc.dram_tensor("wq", (128, KT * D), BF16, kind="ExternalInput")
    wk_d = nc.dram_tensor("wk", (128, KC * D), BF16, kind="ExternalInput")
    wv_d = nc.dram_tensor("wv", (128, KC * D), BF16, kind="ExternalInput")
    wo_d = nc.dram_tensor("wo", (128, KT * D), BF16, kind="ExternalInput")
    bqc_d = nc.dram_tensor("bqc", (128, CT), F32, kind="ExternalInput")
    bk8_d = nc.dram_tensor("bk8", (128, CT), F32, kind="ExternalInput")
    bv_d = nc.dram_tensor("bv", (1, D), BF16, kind="ExternalInput")
    bo128_d = nc.dram_tensor("bo128", (128, D), BF16, kind="ExternalInput")
    sel16_d = nc.dram_tensor("sel16", (SKV, H * 16), BF16, kind="ExternalInput")
    selb_d = nc.dram_tensor("selb", (16, D), BF16, kind="ExternalInput")
    out_d = nc.dram_tensor("out", (SQ, D), F32, kind="ExternalOutput")

    with tile.TileContext(nc) as tc:
        with (
            tc.tile_pool(name="consts", bufs=1) as consts,
            tc.tile_pool(name="wpool", bufs=1) as wpool,
            tc.tile_pool(name="xpool", bufs=2) as xpool,
            tc.tile_pool(name="qpool", bufs=2) as qpool,
            tc.tile_pool(name="epool", bufs=2) as epool,
            tc.tile_pool(name="rp", bufs=2) as rp,
            tc.tile_pool(name="rbpool", bufs=2) as rbpool,
            tc.tile_pool(name="aopool", bufs=2) as aopool,
            tc.tile_pool(name="opool", bufs=3) as opool,
            tc.tile_pool(name="pmm", bufs=2, space="PSUM") as pmm,
            tc.tile_pool(name="psc", bufs=2, space="PSUM") as psc,
            tc.tile_pool(name="ppv", bufs=2, space="PSUM") as ppv,
            tc.tile_pool(name="pnrm", bufs=2, space="PSUM") as pnrm,
        ):
            aoT_tiles = [None, None]
            xT_tiles = {}
            qT_tiles = {}

            def fetch_x(cc, engine=None):
                if cc >= NCH or cc in xT_tiles:
                    return
                t = xpool.tile([128, KT, CH], BF16, tag="xT", name="xT")
                (engine or nc.sync).dma_start(
                    t[:],
                    xt.ap().rearrange("p (kt q) -> p kt q", q=SQ)[
                        :, :, cc * CH:(cc + 1) * CH],
                )
                xT_tiles[cc] = t

            # ---- weights/constants on two parallel HWDGE rings ----
            # ring-sync: wk, xT0, wv, xT1, wo ; ring-act: yt/smalls, wq, rest
            wk_sb = wpool.tile([128, KC, D], BF16, tag="wk")
            nc.sync.dma_start(wk_sb[:], wk_d.ap())
            fetch_x(0)
            wv_sb = wpool.tile([128, KC, D], BF16, tag="wv")
            nc.sync.dma_start(wv_sb[:], wv_d.ap())
            fetch_x(1)
            wo_sb = wpool.tile([128, KT, D], BF16, tag="wo")
            nc.sync.dma_start(wo_sb[:], wo_d.ap())

            yt_sb = consts.tile([128, KC, SKV], BF16, tag="yt")
            nc.scalar.dma_start(yt_sb[:], yt.ap())
            bk8_sb = consts.tile([128, CT], F32, tag="bk8")
            nc.scalar.dma_start(bk8_sb[:], bk8_d.ap())
            bqc_sb = consts.tile([128, CT], F32, tag="bqc")
            nc.scalar.dma_start(bqc_sb[:], bqc_d.ap())
            wq_sb = wpool.tile([128, KT, D], BF16, tag="wq")
            nc.scalar.dma_start(wq_sb[:], wq_d.ap())
            bv_sb = consts.tile([1, D], BF16, tag="bv")
            nc.scalar.dma_start(bv_sb[:], bv_d.ap())
            sel16_sb = consts.tile([SKV, H * 16], BF16, tag="sel16")
            nc.scalar.dma_start(sel16_sb[:], sel16_d.ap())
            selb_sb = consts.tile([16, D], BF16, tag="selb")
            nc.scalar.dma_start(selb_sb[:], selb_d.ap())
            bo128 = consts.tile([128, D], BF16, tag="bo128")
            nc.scalar.dma_start(bo128[:], bo128_d.ap())

            ones77r = consts.tile([1, SKV], BF16, tag="ones77r")
            nc.vector.memset(ones77r[:], 1.0)

            kT_sb = consts.tile([128, CT, SKV], BF16, tag="kT")
            v_sb = consts.tile([SKV, H, DH], BF16, tag="v")

            def emit_qp_tile(cc, ct):
                """Q-proj column-tile ct of chunk cc: 8 matmuls + ACT bias-copy."""
                psq = pmm.tile([128, CH], F32, tag="mm")
                for kt in range(KT):
                    nc.tensor.matmul(
                        psq[:],
                        wq_sb[:, kt, ct * 128:(ct + 1) * 128],
                        xT_tiles[cc][:, kt, :],
                        start=(kt == 0),
                        stop=(kt == KT - 1),
                        skip_group_check=True,
                    )
                nc.scalar.activation(
                    qT_tiles[cc][:, ct, :],
                    psq[:],
                    AF.Identity,
                    bias=bqc_sb[:, ct:ct + 1],
                )
                if ct == CT - 1:
                    xT_tiles.pop(cc)

            def emit_qp_block(cc):
                qT_tiles[cc] = qpool.tile([128, CT, CH], BF16, tag="qT", name="qT")
                for ct in range(CT):
                    emit_qp_tile(cc, ct)

            def emit_op_tile(cc, t):
                """O-proj tile t=(qt*2+n) of chunk cc: 8 matmuls + DVE bias-add + DMA."""
                qt, n = t // 2, t % 2
                aoT_p = aoT_tiles[cc % 2]
                q0 = cc * CH
                pso = pmm.tile([128, CH], F32, tag="mm")
                for kt in range(KT):
                    nc.tensor.matmul(
                        pso[:],
                        aoT_p[:, kt, qt * 128:(qt + 1) * 128],
                        wo_sb[:, kt, n * 512:(n + 1) * 512],
                        start=(kt == 0),
                        stop=(kt == KT - 1),
                        skip_group_check=True,
                    )
                o_sb = opool.tile([128, CH], F32, tag="o")
                nc.vector.tensor_add(o_sb[:], pso[:], bo128[:, n * 512:(n + 1) * 512])
                nc.sync.dma_start(
                    out_d.ap()[q0 + qt * 128: q0 + (qt + 1) * 128,
                               n * 512:(n + 1) * 512],
                    o_sb[:],
                )

            # ---- k projection: kT[c, s] = sum_k wk[k, c] yT[k, s]; fold (.+bk)/8 ----
            for ct in range(CT):
                psk = pmm.tile([128, CH], F32, tag="mm")
                for kt in range(KC):
                    nc.tensor.matmul(
                        psk[:, 0:SKV],
                        wk_sb[:, kt, ct * 128:(ct + 1) * 128],
                        yt_sb[:, kt, :],
                        start=(kt == 0),
                        stop=(kt == KC - 1),
                    )
                nc.scalar.activation(
                    kT_sb[:, ct, :],
                    psk[:, 0:SKV],
                    AF.Identity,
                    scale=0.125,
                    bias=bk8_sb[:, ct:ct + 1],
                )

            # ---- Q-proj(0) (before V-proj: its inputs land earlier) ----
            emit_qp_block(0)

            # ---- v projection: v[s, c] = sum_k yT[k, s] wv[k, c] + bv[c] ----
            for n in range(2):
                psv = pmm.tile([128, CH], F32, tag="mm")
                for kt in range(KC):
                    nc.tensor.matmul(
                        psv[0:SKV, :],
                        yt_sb[:, kt, :],
                        wv_sb[:, kt, n * 512:(n + 1) * 512],
                        start=(kt == 0),
                        stop=False,
                    )
                nc.tensor.matmul(
                    psv[0:SKV, :],
                    ones77r[:],
                    bv_sb[0:1, n * 512:(n + 1) * 512],
                    start=False,
                    stop=True,
                )
                nc.vector.tensor_copy(
                    v_sb[:, n * 8:(n + 1) * 8, :], psv[0:SKV, :]
                )

            # ---- software-pipelined main loop ----
            # iter c: solid Q-proj(c+1) block (c>=1), then attention(c) with
            # fill tiles woven in: O-proj(c-1) tiles (or Q-proj(1) tiles for
            # c==0, which has no prior chunk).
            for c in range(NCH + 1):
                fills = []
                if c == 0:
                    # QP(1) tiles serve as the fills (no prior O-proj yet)
                    qT_tiles[1] = qpool.tile([128, CT, CH], BF16, tag="qT", name="qT")
                    fills = [("qp", 1, ct) for ct in range(CT)]
                elif c <= NCH - 1:
                    if c + 1 <= NCH - 1:
                        emit_qp_block(c + 1)
                    fills = [("op", c - 1, t) for t in range(8)]
                else:
                    for t in range(8):
                        emit_op_tile(c - 1, t)
                    break

                fill_i = [0]

                def drain(upto, _fills=fills, _i=fill_i):
                    while _i[0] < min(upto, len(_fills)):
                        kind, cc, idx = _fills[_i[0]]
                        if kind == "qp":
                            emit_qp_tile(cc, idx)
                        else:
                            emit_op_tile(cc, idx)
                        _i[0] += 1

                fetch_x(c + 2)

                # attention pass A: scores -> exp -> sum-collect [16, CH]
                e_ch = epool.tile([SKV, H, CH], BF16, tag="e")
                ps_sum = pnrm.tile([16, CH], F32, tag="nrm")
                for h in range(H):
                    pssc = psc.tile([SKV, CH], F32, tag="sc")
                    nc.tensor.matmul(
                        pssc[:],
                        kT_sb[(h % 2) * 64:(h % 2) * 64 + 64, h // 2, :],
                        qT_tiles[c][(h % 2) * 64:(h % 2) * 64 + 64, h // 2, :],
                        start=True, stop=True, skip_group_check=True,
                    )
                    nc.scalar.activation(e_ch[:, h, :], pssc[:], AF.Exp)
                    nc.tensor.matmul(
                        ps_sum[:], sel16_sb[:, h * 16:(h + 1) * 16], e_ch[:, h, :],
                        start=(h == 0), stop=(h == H - 1), skip_group_check=True,
                    )
                    if h in (3, 7, 11):
                        drain({3: 1, 7: 2, 11: 3}[h])

                qT_tiles.pop(c)
                r16f = rp.tile([16, CH], F32, tag="rf")
                nc.vector.reciprocal_approx_fast(r16f[:], ps_sum[:])
                r16 = rp.tile([16, CH], BF16, tag="r")
                with nc.allow_low_precision(reason="softmax recip in bf16"):
                    nc.vector.tensor_copy(r16[:], r16f[:])
                # fills here bridge the reciprocal's DVE latency so the PE
                # never idles long enough to trip the HAM re-throttle
                drain(6)

                # pass B: rb = selb^T @ r (broadcast 1/sum to 128 rows), PV,
                # normalize into aoT.
                aoT = aopool.tile([128, KT, CH], BF16, tag="aoT")
                aoT_tiles[c % 2] = aoT
                for hp in range(H // 2):
                    rb_ps = pnrm.tile([128, CH], F32, tag="nrm")
                    nc.tensor.matmul(
                        rb_ps[:],
                        selb_sb[:, hp * 128:(hp + 1) * 128],
                        r16[:],
                        start=True, stop=True, skip_group_check=True,
                    )
                    rb_sb = rbpool.tile([128, CH], F32, tag="rb")
                    nc.scalar.activation(rb_sb[:], rb_ps[:], AF.Identity)
                    pspv = ppv.tile([128, CH], F32, tag="pv")
                    for half in range(2):
                        h = 2 * hp + half
                        nc.tensor.matmul(
                            pspv[half * 64:(half + 1) * 64, :],
                            v_sb[:, h, :],
                            e_ch[:, h, :],
                            start=True, stop=True, skip_group_check=True,
                        )
                    nc.vector.tensor_mul(aoT[:, hp, :], pspv[:], rb_sb[:])
                    if hp in (2, 5):
                        drain({2: 7, 5: 8}[hp])

    nc.compile()
    return nc


def _get_nc():
    if "nc" not in _CACHED:
        _CACHED["nc"] = _build()
    return _CACHED["nc"]


def _prep_inmaps(x, y, wq, bq, wk, bk, wv, bv, wo, bo):
    x = np.asarray(x)
    y = np.asarray(y)
    bf = ml_dtypes.bfloat16

    def tile_w(w, kt):
        # [kt*128, n] -> [128, kt*n] in the device SBUF layout
        w = np.asarray(w)
        n = w.shape[1]
        return np.ascontiguousarray(
            w.reshape(kt, 128, n).transpose(1, 0, 2).reshape(128, kt * n)
        ).astype(bf)

    wq_b = tile_w(wq, KT)
    wk_b = tile_w(wk, KC)
    wv_b = tile_w(wv, KC)
    wo_b = tile_w(wo, KT)
    bv_b = np.asarray(bv).reshape(1, D).astype(bf)
    bo128 = np.ascontiguousarray(
        np.broadcast_to(np.asarray(bo).reshape(1, D), (128, D))).astype(bf)
    bqc = np.ascontiguousarray(
        np.asarray(bq).reshape(CT, 128).T).astype(np.float32)
    bk8 = np.ascontiguousarray(
        (np.asarray(bk).reshape(CT, 128) * 0.125).T).astype(np.float32)
    sel16 = np.zeros((SKV, H, 16), np.float32)
    sel16[:, np.arange(H), np.arange(16)] = 1.0
    sel16 = sel16.reshape(SKV, H * 16).astype(bf)
    selb = np.zeros((16, H // 2, 128), np.float32)
    for hp in range(H // 2):
        selb[2 * hp, hp, 0:64] = 1.0
        selb[2 * hp + 1, hp, 64:128] = 1.0
    selb = selb.reshape(16, D).astype(bf)

    in_maps = []
    for b in range(B):
        in_maps.append({
            "xt": tile_w(x[b].T, KT),
            "yt": tile_w(y[b].T, KC),
            "wq": wq_b, "wk": wk_b, "wv": wv_b, "wo": wo_b,
            "bqc": bqc, "bk8": bk8, "bv": bv_b, "bo128": bo128,
            "sel16": sel16, "selb": selb,
        })
    return in_maps


def kernel(x, y, wq, bq, wk, bk, wv, bv, wo, bo):
    in_maps = _prep_inmaps(x, y, wq, bq, wk, bk, wv, bv, wo, bo)
    nc = _get_nc()
    res = bass_utils.run_bass_kernel_spmd(nc, in_maps, core_ids=list(range(B)))
    out = np.stack([res.results[b]["out"] for b in range(B)], axis=0)
    return out.astype(np.float32)
